# revision 1
# baseline (speedup 1.0000x reference)
"""Trainium2 Bass kernel for nn_AttentionRouter (segment mean-pool + router MLP).

Strategy (data-parallel over the packed token dim, 8 NeuronCores):
  - Each core DMA-streams its 4096-token slice of x ([4096, H*D] fp32, 16 MiB)
    through SBUF and reduces it to per-segment partial feature sums [B, H*D]
    with PE matmuls against a host-built 0/1 segment-membership matrix.
    The membership matrix is an input tensor, so one fixed instruction
    stream handles any (ragged) cu_seq_len.
  - Host gathers the 8 partial [B, H*D] tensors, adds them, divides by
    segment counts, folds heads, and runs the tiny [8,128]->[8,2] router
    MLP + softmax/entropy/broadcast epilogue in fp32 (negligible work).
"""

import numpy as np
from contextlib import ExitStack

import concourse.bass as bass  # noqa: F401  (bass types used via tile/bacc)
import concourse.tile as tile
from concourse import bacc, mybir
from concourse.bass_utils import run_bass_kernel_spmd

# Problem shape (hardcoded per spec: x [32768, 8, 128] fp32, cu_seq_len [9] i32)
T, H, D = 32768, 8, 128
HD = H * D                  # 1024 features per token
B = 8                       # number of segments
NCORES = 8
TPC = T // NCORES           # 4096 tokens per core

# Tunables
G = 4                       # tokens per partition-row within one SBUF tile
FREE = G * HD               # free-dim floats per x tile (G tokens side by side)
NTILES = TPC // (128 * G)   # x DMAs per core
NQ = TPC // 128             # column-groups of 128 tokens (each has B S-columns)
MM_DT = mybir.dt.float32    # matmul dtype (float32 exact; float32r = fast mode)

_built = None
LAST_RESULT = None
_RUN_KWARGS = {}            # test harness may set {"trace": True, ...}


def _build():
    global _built
    if _built is not None:
        return _built

    nc = bacc.Bacc(
        "TRN2", target_bir_lowering=False, debug=False, num_devices=NCORES
    )
    xs = nc.dram_tensor(
        "xs", [NTILES, 128, FREE], mybir.dt.float32, kind="ExternalInput"
    ).ap()
    sm = nc.dram_tensor(
        "sm", [128, NQ * B], mybir.dt.float32, kind="ExternalInput"
    ).ap()
    out = nc.dram_tensor(
        "partial", [B, HD], mybir.dt.float32, kind="ExternalOutput"
    ).ap()

    with tile.TileContext(nc) as tc:
        with ExitStack() as ctx:
            spool = ctx.enter_context(tc.tile_pool(name="spool", bufs=1))
            xpool = ctx.enter_context(tc.tile_pool(name="xpool", bufs=NTILES))
            ppool = ctx.enter_context(
                tc.tile_pool(name="psum", bufs=1, space="PSUM")
            )
            opool = ctx.enter_context(tc.tile_pool(name="opool", bufs=1))

            s_t = spool.tile([128, NQ * B], mybir.dt.float32)
            nc.scalar.dma_start(out=s_t[:], in_=sm[:])

            acc = ppool.tile([B, HD], mybir.dt.float32)
            for t in range(NTILES):
                xt = xpool.tile([128, FREE], mybir.dt.float32)
                eng = nc.sync if (t % 2 == 0) else nc.scalar
                eng.dma_start(out=xt[:], in_=xs[t])
                for g in range(G):
                    q = t * G + g
                    lhsT = s_t[:, q * B : (q + 1) * B].bitcast(MM_DT)
                    for h in range(2):
                        nc.tensor.matmul(
                            acc[:, h * 512 : (h + 1) * 512],
                            lhsT,
                            xt[
                                :, g * HD + h * 512 : g * HD + (h + 1) * 512
                            ].bitcast(MM_DT),
                            start=(q == 0),
                            stop=(q == NQ - 1),
                        )

            o_t = opool.tile([B, HD], mybir.dt.float32)
            nc.vector.tensor_copy(o_t[:, 0:512], acc[:, 0:512])
            nc.vector.tensor_copy(o_t[:, 512:1024], acc[:, 512:1024])
            nc.sync.dma_start(out=out[:], in_=o_t[:])

    nc.compile()
    _built = nc
    return nc


def _membership(cu: np.ndarray, core: int) -> np.ndarray:
    """[128, NQ*B] fp32: S[p, q*B+b] = 1 iff token(core,q,p) in segment b.

    Token layout must match the SBUF tile layout the kernel reads:
    tile t, partition p, column-group g holds token t*128*G + p*G + g.
    """
    q = np.arange(NQ)
    p = np.arange(128)
    tok = core * TPC + (q[None, :] // G) * (128 * G) + p[:, None] * G + (q[None, :] % G)
    memb = (tok[:, :, None] >= cu[None, None, :-1]) & (
        tok[:, :, None] < cu[None, None, 1:]
    )
    return np.ascontiguousarray(memb.astype(np.float32).reshape(128, NQ * B))


def _silu(v):
    return v * (1.0 / (1.0 + np.exp(-v)))


def kernel(x, cu_seq_len, w1, b1, w2, b2, w3, b3, w4, b4, w5, b5):
    global LAST_RESULT
    nc = _build()

    x = np.ascontiguousarray(np.asarray(x, dtype=np.float32))
    cu = np.asarray(cu_seq_len).astype(np.int64)
    xs_flat = x.reshape(T, HD)

    in_maps = []
    for m in range(NCORES):
        shard = np.ascontiguousarray(
            xs_flat[m * TPC : (m + 1) * TPC].reshape(NTILES, 128, FREE)
        )
        in_maps.append({"xs": shard, "sm": _membership(cu, m)})

    res = run_bass_kernel_spmd(nc, in_maps, list(range(NCORES)), **_RUN_KWARGS)
    LAST_RESULT = res
    partials = np.stack(
        [res.results[i]["partial"] for i in range(NCORES)]
    )  # [NCORES, B, HD]

    sums = partials.sum(axis=0, dtype=np.float32)           # [B, HD]
    counts = (cu[1:] - cu[:-1]).astype(np.float32)          # [B]
    with np.errstate(divide="ignore", invalid="ignore"):
        pooled = sums / counts[:, None]                     # [B, HD]
    plm = pooled.reshape(B, H, D).mean(axis=1).astype(np.float32)  # [B, D]

    w1 = np.asarray(w1, np.float32); b1 = np.asarray(b1, np.float32)
    w2 = np.asarray(w2, np.float32); b2 = np.asarray(b2, np.float32)
    w3 = np.asarray(w3, np.float32); b3 = np.asarray(b3, np.float32)
    w4 = np.asarray(w4, np.float32); b4 = np.asarray(b4, np.float32)
    w5 = np.asarray(w5, np.float32); b5 = np.asarray(b5, np.float32)

    pooled_hidden = _silu(plm @ w1 + b1) @ w2 + b2          # [B, 2D]
    logits = _silu(_silu(pooled_hidden @ w3 + b3) @ w4 + b4) @ w5 + b5  # [B, 2]
    pooled_hidden = pooled_hidden.astype(np.float32)
    logits = logits.astype(np.float32)

    zmax = logits.max(axis=-1, keepdims=True)
    ez = np.exp(logits - zmax)
    z_soft_full = (ez / ez.sum(axis=-1, keepdims=True)).astype(np.float32)
    z_hard_full = np.eye(2, dtype=np.float32)[np.argmax(z_soft_full, axis=-1)]
    z = (z_hard_full + z_soft_full - z_soft_full)[:, 1:2]
    z_soft = z_soft_full[:, 1:2]
    eps = np.float32(1e-8)
    entropy = np.float32(-(z_soft * np.log(z_soft + eps)).mean())

    pooled_hidden_expanded = np.ascontiguousarray(
        np.broadcast_to(pooled_hidden[:, None, :], (B, H, pooled_hidden.shape[-1]))
    )
    z_soft_expanded = np.ascontiguousarray(np.broadcast_to(z_soft, (B, H)))
    z_hard_return = np.ascontiguousarray(
        np.broadcast_to(z_hard_full[:, None, :], (B, H, 2))
    )
    z_expanded = np.ascontiguousarray(np.broadcast_to(z, (B, H))[:, :, None])
    logits_expanded = np.ascontiguousarray(
        np.broadcast_to(logits[:, None, :], (B, H, 2))
    )

    return (
        pooled_hidden_expanded,
        z_soft_expanded,
        z_hard_return,
        z_expanded,
        logits_expanded,
        entropy,
    )


# revision 5
# speedup vs baseline: 1.2481x; 1.2481x over previous
"""Trainium2 Bass kernel for nn_AttentionRouter (segment mean-pool + router MLP).

Strategy (data-parallel over the packed token dim, 8 NeuronCores):
  - Each core DMA-streams its 4096-token slice of x ([4096, H*D] fp32, 16 MiB)
    through SBUF and reduces it to per-segment partial feature sums [B, H*D]
    with PE matmuls against a host-built 0/1 segment-membership matrix.
    The membership matrix is an input tensor, so one fixed instruction
    stream handles any (ragged) cu_seq_len.
  - Host gathers the 8 partial [B, H*D] tensors, adds them, divides by
    segment counts, folds heads, and runs the tiny [8,128]->[8,2] router
    MLP + softmax/entropy/broadcast epilogue in fp32 (negligible work).
"""

import numpy as np
from contextlib import ExitStack

import concourse.bass as bass  # noqa: F401  (bass types used via tile/bacc)
import concourse.tile as tile
from concourse import bacc, mybir
from concourse.bass_utils import run_bass_kernel_spmd

# Problem shape (hardcoded per spec: x [32768, 8, 128] fp32, cu_seq_len [9] i32)
T, H, D = 32768, 8, 128
HD = H * D                  # 1024 features per token
B = 8                       # number of segments
NCORES = 8
TPC = T // NCORES           # 4096 tokens per core

# Tunables
G = 4                       # tokens per partition-row within one SBUF tile
FREE = G * HD               # free-dim floats per x tile (G tokens side by side)
NTILES = TPC // (128 * G)   # x DMAs per core
NQ = TPC // 128             # column-groups of 128 tokens (each has B S-columns)
MM_DT = mybir.dt.float32r   # matmul dtype (float32 exact; float32r = fast mode)

_built = None
LAST_RESULT = None
_RUN_KWARGS = {}            # test harness may set {"trace": True, ...}


def _build():
    global _built
    if _built is not None:
        return _built

    nc = bacc.Bacc(
        "TRN2", target_bir_lowering=False, debug=False, num_devices=NCORES
    )
    xs = nc.dram_tensor(
        "xs", [NTILES, 128, FREE], MM_DT, kind="ExternalInput"
    ).ap()
    sm = nc.dram_tensor(
        "sm", [128, NQ * B], MM_DT, kind="ExternalInput"
    ).ap()
    out = nc.dram_tensor(
        "partial", [B, HD], mybir.dt.float32, kind="ExternalOutput"
    ).ap()

    with tile.TileContext(nc) as tc:
        with ExitStack() as ctx:
            spool = ctx.enter_context(tc.tile_pool(name="spool", bufs=1))
            xpool = ctx.enter_context(tc.tile_pool(name="xpool", bufs=NTILES))
            ppool = ctx.enter_context(
                tc.tile_pool(name="psum", bufs=1, space="PSUM")
            )
            opool = ctx.enter_context(tc.tile_pool(name="opool", bufs=1))

            s_t = spool.tile([128, NQ * B], MM_DT)
            nc.scalar.dma_start(out=s_t[:], in_=sm[:])

            acc = ppool.tile([B, HD], mybir.dt.float32)
            for t in range(NTILES):
                xt = xpool.tile([128, FREE], MM_DT)
                eng = nc.sync if (t % 2 == 0) else nc.scalar
                eng.dma_start(out=xt[:], in_=xs[t])
                for g in range(G):
                    q = t * G + g
                    lhsT = s_t[:, q * B : (q + 1) * B]
                    for h in range(2):
                        nc.tensor.matmul(
                            acc[:, h * 512 : (h + 1) * 512],
                            lhsT,
                            xt[:, g * HD + h * 512 : g * HD + (h + 1) * 512],
                            start=(q == 0),
                            stop=(q == NQ - 1),
                        )

            o_t = opool.tile([B, HD], mybir.dt.float32)
            nc.vector.tensor_copy(o_t[:, 0:512], acc[:, 0:512])
            nc.vector.tensor_copy(o_t[:, 512:1024], acc[:, 512:1024])
            nc.sync.dma_start(out=out[:], in_=o_t[:])

    nc.compile()
    _built = nc
    return nc


def _membership(cu: np.ndarray, core: int) -> np.ndarray:
    """[128, NQ*B] fp32: S[p, q*B+b] = 1 iff token(core,q,p) in segment b.

    Token layout must match the SBUF tile layout the kernel reads:
    tile t, partition p, column-group g holds token t*128*G + p*G + g.
    """
    q = np.arange(NQ)
    p = np.arange(128)
    tok = core * TPC + (q[None, :] // G) * (128 * G) + p[:, None] * G + (q[None, :] % G)
    memb = (tok[:, :, None] >= cu[None, None, :-1]) & (
        tok[:, :, None] < cu[None, None, 1:]
    )
    return np.ascontiguousarray(memb.astype(np.float32).reshape(128, NQ * B))


def _silu(v):
    return v * (1.0 / (1.0 + np.exp(-v)))


def _round_fp32r(a: np.ndarray) -> np.ndarray:
    """Round fp32 values to the fp32r encoding (mantissa trimmed to 11
    bits, round-to-nearest) — the PE's fast-fp32 matmul input format."""
    u = a.view(np.uint32)
    r = (u + np.uint32(0x800)) & np.uint32(0xFFFFF000)
    return r.view(np.float32)


def kernel(x, cu_seq_len, w1, b1, w2, b2, w3, b3, w4, b4, w5, b5):
    global LAST_RESULT
    nc = _build()

    x = np.ascontiguousarray(np.asarray(x, dtype=np.float32))
    cu = np.asarray(cu_seq_len).astype(np.int64)
    if MM_DT == mybir.dt.float32r:
        x = _round_fp32r(x)
    xs_flat = x.reshape(T, HD)

    in_maps = []
    for m in range(NCORES):
        shard = np.ascontiguousarray(
            xs_flat[m * TPC : (m + 1) * TPC].reshape(NTILES, 128, FREE)
        )
        in_maps.append({"xs": shard, "sm": _membership(cu, m)})

    res = run_bass_kernel_spmd(nc, in_maps, list(range(NCORES)), **_RUN_KWARGS)
    LAST_RESULT = res
    partials = np.stack(
        [res.results[i]["partial"] for i in range(NCORES)]
    )  # [NCORES, B, HD]

    sums = partials.sum(axis=0, dtype=np.float32)           # [B, HD]
    counts = (cu[1:] - cu[:-1]).astype(np.float32)          # [B]
    with np.errstate(divide="ignore", invalid="ignore"):
        pooled = sums / counts[:, None]                     # [B, HD]
    plm = pooled.reshape(B, H, D).mean(axis=1).astype(np.float32)  # [B, D]

    w1 = np.asarray(w1, np.float32); b1 = np.asarray(b1, np.float32)
    w2 = np.asarray(w2, np.float32); b2 = np.asarray(b2, np.float32)
    w3 = np.asarray(w3, np.float32); b3 = np.asarray(b3, np.float32)
    w4 = np.asarray(w4, np.float32); b4 = np.asarray(b4, np.float32)
    w5 = np.asarray(w5, np.float32); b5 = np.asarray(b5, np.float32)

    pooled_hidden = _silu(plm @ w1 + b1) @ w2 + b2          # [B, 2D]
    logits = _silu(_silu(pooled_hidden @ w3 + b3) @ w4 + b4) @ w5 + b5  # [B, 2]
    pooled_hidden = pooled_hidden.astype(np.float32)
    logits = logits.astype(np.float32)

    zmax = logits.max(axis=-1, keepdims=True)
    ez = np.exp(logits - zmax)
    z_soft_full = (ez / ez.sum(axis=-1, keepdims=True)).astype(np.float32)
    z_hard_full = np.eye(2, dtype=np.float32)[np.argmax(z_soft_full, axis=-1)]
    z = (z_hard_full + z_soft_full - z_soft_full)[:, 1:2]
    z_soft = z_soft_full[:, 1:2]
    eps = np.float32(1e-8)
    entropy = np.float32(-(z_soft * np.log(z_soft + eps)).mean())

    pooled_hidden_expanded = np.ascontiguousarray(
        np.broadcast_to(pooled_hidden[:, None, :], (B, H, pooled_hidden.shape[-1]))
    )
    z_soft_expanded = np.ascontiguousarray(np.broadcast_to(z_soft, (B, H)))
    z_hard_return = np.ascontiguousarray(
        np.broadcast_to(z_hard_full[:, None, :], (B, H, 2))
    )
    z_expanded = np.ascontiguousarray(np.broadcast_to(z, (B, H))[:, :, None])
    logits_expanded = np.ascontiguousarray(
        np.broadcast_to(logits[:, None, :], (B, H, 2))
    )

    return (
        pooled_hidden_expanded,
        z_soft_expanded,
        z_hard_return,
        z_expanded,
        logits_expanded,
        entropy,
    )


# revision 7
# speedup vs baseline: 1.4132x; 1.1323x over previous
"""Trainium2 Bass kernel for nn_AttentionRouter (segment mean-pool + router MLP).

Strategy (data-parallel over the packed token dim, 8 NeuronCores):
  - Each core DMA-streams its 4096-token slice of x ([4096, H*D] fp32, 16 MiB)
    through SBUF and reduces it to per-segment partial feature sums [B, H*D]
    with PE matmuls against a host-built 0/1 segment-membership matrix.
    The membership matrix is an input tensor, so one fixed instruction
    stream handles any (ragged) cu_seq_len.
  - Host gathers the 8 partial [B, H*D] tensors, adds them, divides by
    segment counts, folds heads, and runs the tiny [8,128]->[8,2] router
    MLP + softmax/entropy/broadcast epilogue in fp32 (negligible work).
"""

import numpy as np
from contextlib import ExitStack

import concourse.bass as bass  # noqa: F401  (bass types used via tile/bacc)
import concourse.tile as tile
from concourse import bacc, mybir
from concourse.bass_utils import run_bass_kernel_spmd

# Problem shape (hardcoded per spec: x [32768, 8, 128] fp32, cu_seq_len [9] i32)
T, H, D = 32768, 8, 128
HD = H * D                  # 1024 features per token
B = 8                       # number of segments
NCORES = 8
TPC = T // NCORES           # 4096 tokens per core

# Tunables
import os as _os
G = int(_os.environ.get("KG", "4"))  # tokens per partition-row per SBUF tile
FREE = G * HD               # free-dim floats per x tile (G tokens side by side)
NTILES = TPC // (128 * G)   # x DMAs per core
NQ = TPC // 128             # column-groups of 128 tokens (each has B S-columns)
MM_DT = mybir.dt.float32r   # matmul dtype (float32 exact; float32r = fast mode)

_built = None
LAST_RESULT = None
_RUN_KWARGS = {}            # test harness may set {"trace": True, ...}


def _build():
    global _built
    if _built is not None:
        return _built

    nc = bacc.Bacc(
        "TRN2",
        target_bir_lowering=False,
        debug=False,
        enable_asserts=False,
        num_devices=NCORES,
    )
    xs = nc.dram_tensor(
        "xs", [NTILES, 128, FREE], MM_DT, kind="ExternalInput"
    ).ap()
    sm = nc.dram_tensor(
        "sm", [128, NQ * B], MM_DT, kind="ExternalInput"
    ).ap()
    out = nc.dram_tensor(
        "partial", [B, HD], mybir.dt.float32, kind="ExternalOutput"
    ).ap()

    with tile.TileContext(nc) as tc:
        with ExitStack() as ctx:
            spool = ctx.enter_context(tc.tile_pool(name="spool", bufs=1))
            xpool = ctx.enter_context(tc.tile_pool(name="xpool", bufs=NTILES))
            ppool = ctx.enter_context(
                tc.tile_pool(name="psum", bufs=1, space="PSUM")
            )
            opool = ctx.enter_context(tc.tile_pool(name="opool", bufs=1))

            s_t = spool.tile([128, NQ * B], MM_DT)
            nc.scalar.dma_start(out=s_t[:], in_=sm[:])

            acc = ppool.tile([B, HD], mybir.dt.float32)
            for t in range(NTILES):
                xt = xpool.tile([128, FREE], MM_DT)
                eng = nc.sync if (t % 2 == 0) else nc.scalar
                eng.dma_start(out=xt[:], in_=xs[t])
                for g in range(G):
                    q = t * G + g
                    lhsT = s_t[:, q * B : (q + 1) * B]
                    for h in range(2):
                        nc.tensor.matmul(
                            acc[:, h * 512 : (h + 1) * 512],
                            lhsT,
                            xt[:, g * HD + h * 512 : g * HD + (h + 1) * 512],
                            start=(q == 0),
                            stop=(q == NQ - 1),
                        )

            o_t = opool.tile([B, HD], mybir.dt.float32)
            nc.vector.tensor_copy(o_t[:, 0:512], acc[:, 0:512])
            nc.vector.tensor_copy(o_t[:, 512:1024], acc[:, 512:1024])
            nc.sync.dma_start(out=out[:], in_=o_t[:])

    nc.compile()
    _built = nc
    return nc


def _membership(cu: np.ndarray, core: int) -> np.ndarray:
    """[128, NQ*B] fp32: S[p, q*B+b] = 1 iff token(core,q,p) in segment b.

    Token layout must match the SBUF tile layout the kernel reads:
    tile t, partition p, column-group g holds token t*128*G + p*G + g.
    """
    q = np.arange(NQ)
    p = np.arange(128)
    tok = core * TPC + (q[None, :] // G) * (128 * G) + p[:, None] * G + (q[None, :] % G)
    memb = (tok[:, :, None] >= cu[None, None, :-1]) & (
        tok[:, :, None] < cu[None, None, 1:]
    )
    return np.ascontiguousarray(memb.astype(np.float32).reshape(128, NQ * B))


def _silu(v):
    return v * (1.0 / (1.0 + np.exp(-v)))


def _round_fp32r(a: np.ndarray) -> np.ndarray:
    """Round fp32 values to the fp32r encoding (mantissa trimmed to 11
    bits, round-to-nearest) — the PE's fast-fp32 matmul input format."""
    u = a.view(np.uint32)
    r = (u + np.uint32(0x800)) & np.uint32(0xFFFFF000)
    return r.view(np.float32)


def kernel(x, cu_seq_len, w1, b1, w2, b2, w3, b3, w4, b4, w5, b5):
    global LAST_RESULT
    nc = _build()

    x = np.ascontiguousarray(np.asarray(x, dtype=np.float32))
    cu = np.asarray(cu_seq_len).astype(np.int64)
    if MM_DT == mybir.dt.float32r:
        x = _round_fp32r(x)
    xs_flat = x.reshape(T, HD)

    in_maps = []
    for m in range(NCORES):
        shard = np.ascontiguousarray(
            xs_flat[m * TPC : (m + 1) * TPC].reshape(NTILES, 128, FREE)
        )
        in_maps.append({"xs": shard, "sm": _membership(cu, m)})

    res = run_bass_kernel_spmd(nc, in_maps, list(range(NCORES)), **_RUN_KWARGS)
    LAST_RESULT = res
    partials = np.stack(
        [res.results[i]["partial"] for i in range(NCORES)]
    )  # [NCORES, B, HD]

    sums = partials.sum(axis=0, dtype=np.float32)           # [B, HD]
    counts = (cu[1:] - cu[:-1]).astype(np.float32)          # [B]
    with np.errstate(divide="ignore", invalid="ignore"):
        pooled = sums / counts[:, None]                     # [B, HD]
    plm = pooled.reshape(B, H, D).mean(axis=1).astype(np.float32)  # [B, D]

    w1 = np.asarray(w1, np.float32); b1 = np.asarray(b1, np.float32)
    w2 = np.asarray(w2, np.float32); b2 = np.asarray(b2, np.float32)
    w3 = np.asarray(w3, np.float32); b3 = np.asarray(b3, np.float32)
    w4 = np.asarray(w4, np.float32); b4 = np.asarray(b4, np.float32)
    w5 = np.asarray(w5, np.float32); b5 = np.asarray(b5, np.float32)

    pooled_hidden = _silu(plm @ w1 + b1) @ w2 + b2          # [B, 2D]
    logits = _silu(_silu(pooled_hidden @ w3 + b3) @ w4 + b4) @ w5 + b5  # [B, 2]
    pooled_hidden = pooled_hidden.astype(np.float32)
    logits = logits.astype(np.float32)

    zmax = logits.max(axis=-1, keepdims=True)
    ez = np.exp(logits - zmax)
    z_soft_full = (ez / ez.sum(axis=-1, keepdims=True)).astype(np.float32)
    z_hard_full = np.eye(2, dtype=np.float32)[np.argmax(z_soft_full, axis=-1)]
    z = (z_hard_full + z_soft_full - z_soft_full)[:, 1:2]
    z_soft = z_soft_full[:, 1:2]
    eps = np.float32(1e-8)
    entropy = np.float32(-(z_soft * np.log(z_soft + eps)).mean())

    pooled_hidden_expanded = np.ascontiguousarray(
        np.broadcast_to(pooled_hidden[:, None, :], (B, H, pooled_hidden.shape[-1]))
    )
    z_soft_expanded = np.ascontiguousarray(np.broadcast_to(z_soft, (B, H)))
    z_hard_return = np.ascontiguousarray(
        np.broadcast_to(z_hard_full[:, None, :], (B, H, 2))
    )
    z_expanded = np.ascontiguousarray(np.broadcast_to(z, (B, H))[:, :, None])
    logits_expanded = np.ascontiguousarray(
        np.broadcast_to(logits[:, None, :], (B, H, 2))
    )

    return (
        pooled_hidden_expanded,
        z_soft_expanded,
        z_hard_return,
        z_expanded,
        logits_expanded,
        entropy,
    )
